# revision 1
# baseline (speedup 1.0000x reference)
"""Chebyshev approximation kernel for Trainium2 (8 NeuronCores, SPMD data-parallel).

Math: reference computes
    y_at_nodes = (1-t) * y[:, idx] + t * y[:, idx+1]      # [n_obs, deg]
    out        = (y_at_nodes @ basis).reshape(-1)         # [n_obs*deg]
Both steps are linear in y, so we fold them into a single matrix on host:
    C[k, d] = sum_j W[k, j] * basis[j, d],   W = interp weights (2 nnz/col)
    out     = y @ C          # [n_obs, 2049] @ [2049, 1024]
The device kernel is one GEMM per 128-row block: PE-transpose the y block
(grid axis onto partitions, float32r transpose-mode matmuls into PSUM, drained
by wide DVE/ACT copies), then 16 accumulating float32r matmuls (k-tiles of
128) per 512-wide output half; grid column 2048's rank-1 contribution is
folded on DVE during the output copy. float32r = fp32 storage with FP22
multiplies at full PE rate (1 cycle/row for N>=256), fp32 accumulation.

Sharding: y rows split 8192/core across 8 cores; C replicated.
"""

import os
import numpy as np

DEG = 1024
N_OBS = 65536
M_P1 = 2049
N_CORES = 8
ROWS_PER_CORE = N_OBS // N_CORES  # 8192
KT = 17                           # contraction tiles of 128 (2049 -> 2176 padded)
KP = KT * 128                     # 2176
RB = 128                          # rows per block

_COMPILED = {}
LAST_RESULTS = None


def _cheb_c_matrix(x: np.ndarray) -> np.ndarray:
    """C [KP, DEG] float32 with zero pad rows >= 2049; out = y @ C[:M_P1]."""
    x = np.asarray(x, dtype=np.float32)
    k = np.arange(DEG, dtype=np.float32)
    # float32 node computation, mimicking the jax reference
    ang = (np.float32(np.pi) * (k + np.float32(0.5))) / np.float32(DEG)
    nodes = np.sort(np.cos(ang.astype(np.float32)).astype(np.float32))
    norm = ((np.float32(2.0) - (k == 0).astype(np.float32)) / np.float32(DEG)).astype(
        np.float64
    )
    # basis[j, d] = norm_d * cos(d * arccos(node_j)); f64 from f32 nodes
    theta = np.arccos(nodes.astype(np.float64))
    basis = norm[None, :] * np.cos(k.astype(np.float64)[None, :] * theta[:, None])
    idx = np.clip(np.searchsorted(x, nodes, side="right") - 1, 0, M_P1 - 2)
    a = x[idx]
    b = x[idx + 1]
    t = ((nodes - a) / (b - a)).astype(np.float64)
    C = np.zeros((KP, DEG), dtype=np.float64)
    np.add.at(C, idx, (1.0 - t)[:, None] * basis)
    np.add.at(C, idx + 1, t[:, None] * basis)
    return np.ascontiguousarray(C.astype(np.float32))


def build_cheb_kernel(tc, y_ap, c_ap, id_ap, o_ap, rows):
    """Emit the per-core program: out[rows, DEG] = y[rows, M_P1] @ C[:M_P1]."""
    import concourse.mybir as mybir

    nc = tc.nc
    f32 = mybir.dt.float32
    f32r = mybir.dt.float32r
    nblocks = rows // RB

    # 16 full k-tiles cover columns 0..2047; column 2048's rank-1 update is
    # folded on DVE during the output copy (out += y[:,2048] * C[2048,:]).
    KTM = 16
    # Stages grouped 4-per-PSUM-bank: a burst of 4 PE transposes shares one
    # PSUM bank, drained by a single wide copy; main matmuls run one group
    # behind so the drain is off their critical path.
    G = 4

    with (
        tc.tile_pool(name="consts", bufs=1) as consts,
        tc.tile_pool(name="ypool", bufs=4) as ypool,
        tc.tile_pool(name="ytpool", bufs=2) as ytpool,
        tc.tile_pool(name="opool", bufs=3) as opool,
        tc.tile_pool(name="pst", bufs=4, space="PSUM") as pstp,
        tc.tile_pool(name="pso", bufs=2, space="PSUM") as psop,
    ):
        ident = consts.tile([128, 128], f32r)
        nc.sync.dma_start(out=ident, in_=id_ap)
        # C resident in SBUF: [partition-within-tile, ktile, d]; chunked DMAs
        # on the scalar HWDGE queue so y loads (sync queue) aren't blocked.
        # Alternate C chunks between the scalar and sync HWDGE queues:
        # serialized on one queue the 16 chunks take ~24us and the first
        # blocks' matmuls stall waiting for late k-tiles. (gpsimd SWDGE is
        # avoided — its ring setup adds ~5us to engine startup.)
        c_sb = consts.tile([128, KTM, DEG], f32r)
        c_r = c_ap.rearrange("(t p) n -> p t n", p=128)
        def load_c(k):
            eng = nc.scalar if k % 2 == 0 else nc.sync
            eng.dma_start(out=c_sb[:, k, :], in_=c_r[:, k, :])
        # C row 2048 replicated across partitions for the DVE rank-1 fold.
        c_rep = consts.tile([128, DEG], f32)
        import concourse.bass as bass

        c_row = c_ap[KTM * 128 : KTM * 128 + 1, :].bitcast(f32)
        c_row_bc = bass.AP(
            tensor=c_row.tensor, offset=c_row.offset, ap=[[0, 128]] + list(c_row.ap[1:])
        )

        ybs, ytbs, pss = {}, {}, {}

        def load_y(b, split=False):
            yb = ypool.tile([128, M_P1], f32r, name="yb", tag="yb")
            rows = y_ap[b * RB : (b + 1) * RB, :]
            if split:
                # halves so block 0's first transposes start sooner
                nc.sync.dma_start(out=yb[:, 0:1024], in_=rows[:, 0:1024])
                nc.sync.dma_start(out=yb[:, 1024:M_P1], in_=rows[:, 1024:M_P1])
            else:
                nc.sync.dma_start(out=yb, in_=rows)
            ybs[b] = yb

        def emit_t_group(b, g):
            if g == 0:
                ytbs[b] = ytpool.tile([128, KTM, 128], f32r, name="ytb", tag="ytb")
            pst = pstp.tile([128, G, 128], f32r, name="pst", tag="pst")
            for j in range(G):
                k = g * G + j
                nc.tensor.transpose(
                    pst[:, j, :], ybs[b][:, k * 128 : (k + 1) * 128], ident
                )
            dst = ytbs[b][:, g * G : (g + 1) * G, :]
            if g % 2 == 0:
                nc.vector.tensor_copy(dst, pst)
            else:
                nc.scalar.copy(dst, pst)

        def emit_m_group(b, g):
            if g == 0:
                pss[b] = psop.tile([128, DEG], f32, name="ps", tag="ps")
            ps = pss[b]
            for j in range(G):
                k = g * G + j
                for nh in range(2):
                    nc.tensor.matmul(
                        ps[:, nh * 512 : (nh + 1) * 512],
                        ytbs[b][:, k, :],
                        c_sb[:, k, nh * 512 : (nh + 1) * 512],
                        start=(k == 0),
                        stop=(k == KTM - 1),
                    )
            if g == KTM // G - 1:
                tmp = opool.tile([128, DEG], f32, name="tmp", tag="tmp")
                nc.vector.tensor_scalar_mul(
                    tmp, c_rep, ybs[b][:, 2048:2049].bitcast(f32)
                )
                osb = opool.tile([128, DEG], f32, name="osb", tag="osb")
                nc.vector.tensor_add(osb, ps, tmp)
                nc.scalar.dma_start(out=o_ap[b * RB : (b + 1) * RB, :], in_=osb)
                del ybs[b], ytbs[b], pss[b]

        groups = [(b, g) for b in range(nblocks) for g in range(KTM // G)]
        load_y(0, split=True)
        for k in range(KTM):
            load_c(k)
        nc.scalar.dma_start(out=c_rep, in_=c_row_bc)
        for i in range(len(groups) + 1):
            if i < len(groups):
                b, g = groups[i]
                if g == 0 and b + 1 < nblocks:
                    load_y(b + 1)
                emit_t_group(b, g)
            if i >= 1:
                emit_m_group(*groups[i - 1])


def _build_nc(rows):
    import concourse.mybir as mybir
    import concourse.tile as tile
    from concourse import bacc

    f32 = mybir.dt.float32
    f32r = mybir.dt.float32r
    nc = bacc.Bacc(
        "TRN2",
        target_bir_lowering=False,
        debug=False,
        enable_asserts=False,
        num_devices=N_CORES,
    )
    y_ap = nc.dram_tensor("y", [rows, M_P1], f32r, kind="ExternalInput").ap()
    c_ap = nc.dram_tensor("c", [KP, DEG], f32r, kind="ExternalInput").ap()
    id_ap = nc.dram_tensor("ident", [128, 128], f32r, kind="ExternalInput").ap()
    o_ap = nc.dram_tensor("o", [rows, DEG], f32, kind="ExternalOutput").ap()
    with tile.TileContext(nc) as tc:
        build_cheb_kernel(tc, y_ap, c_ap, id_ap, o_ap, rows)
    nc.compile()
    return nc


def _get_compiled(rows=ROWS_PER_CORE):
    if rows not in _COMPILED:
        _COMPILED[rows] = _build_nc(rows)
    return _COMPILED[rows]


def kernel(x: np.ndarray, y: np.ndarray) -> np.ndarray:
    global LAST_RESULTS
    from concourse import bass_utils

    x = np.asarray(x, dtype=np.float32)
    y = np.ascontiguousarray(np.asarray(y, dtype=np.float32))
    assert y.shape == (N_OBS, M_P1), y.shape
    C = _cheb_c_matrix(x)

    nc = _get_compiled()
    ident = np.ascontiguousarray(np.eye(128, dtype=np.float32))
    in_maps = [
        {"y": y[i * ROWS_PER_CORE : (i + 1) * ROWS_PER_CORE], "c": C, "ident": ident}
        for i in range(N_CORES)
    ]
    trace = bool(int(os.environ.get("CHEB_TRACE", "0")))
    res = bass_utils.run_bass_kernel_spmd(
        nc, in_maps, core_ids=list(range(N_CORES)), trace=trace
    )
    LAST_RESULTS = res
    out = np.concatenate([res.results[i]["o"] for i in range(N_CORES)], axis=0)
    return out.reshape(-1)



# revision 15
# speedup vs baseline: 1.5011x; 1.5011x over previous
"""Chebyshev approximation kernel for Trainium2 (8 NeuronCores, SPMD data-parallel).

Math: reference computes
    z        = interp(y at Chebyshev nodes)            # [n_obs, 1024]
    out      = (z @ basis).reshape(-1)                 # DCT-type transform

v2: instead of folding interp into one dense [2049,1024] GEMM (baseline,
~17.2 GMAC/core), exploit (a) the 2-nnz-per-node sparsity of the interp and
(b) the node symmetry node_k = -node_{1023-k}, which makes the first DCT
butterfly fold free:
  stage 1: z' = y @ Wz'   as 16 BANDED matmuls (~1030 moving cols total,
           z' column order chosen so each 128-row grid tile maps to one
           contiguous column range, lower/upper grid halves hitting
           disjoint PSUM banks; per-element has_written gives
           write-or-accumulate for the small band overlaps)
  fold:    u = z'lo + z'hi (even coeffs), v = z'lo - z'hi (odd) on DVE
  stage 2: out_even = u @ Be (DCT-II-512), out_odd = v @ Bo, both dense
           [512,512] in bf16, fp32 PSUM accum; interleaved on drain.
All PE inputs bf16 (fast weight load); ~4.3 GMAC/core on PE vs 17.2 baseline.
Grid row 2048 (only the +1-most node's interval reaches it) is folded in as
a rank-1 DVE fix on z' column 0.

Sharding: y rows split 8192/core across 8 cores; tables replicated.
"""

import os
import numpy as np
import ml_dtypes

BF16 = ml_dtypes.bfloat16

DEG = 1024
H = DEG // 2            # 512
N_OBS = 65536
M_P1 = 2049
N_CORES = 8
ROWS_PER_CORE = N_OBS // N_CORES  # 8192
RB = 128                # rows per block
NKT = 16                # grid k-tiles of 128 (rows 0..2047; row 2048 special)

_COMPILED = {}
_TABLES = None
LAST_RESULTS = None


def _build_tables(x: np.ndarray):
    """Host tables: packed banded interp Wz' (bf16), band metadata, Be/Bo."""
    x = np.asarray(x, dtype=np.float32)
    k = np.arange(DEG, dtype=np.float32)
    ang = (np.float32(np.pi) * (k + np.float32(0.5))) / np.float32(DEG)
    nodes = np.sort(np.cos(ang).astype(np.float32))
    idx = np.clip(np.searchsorted(x, nodes, side="right") - 1, 0, M_P1 - 2)
    a = x[idx]
    b = x[idx + 1]
    t = ((nodes - a) / (b - a)).astype(np.float64)

    # z' column of (ascending) node j: nodes j>=512 -> col 1023-j (so col c
    # has theta = pi(c+.5)/1024), nodes j<512 -> col 512+j (theta = pi-phi_c).
    j = np.arange(DEG)
    col = np.where(j >= H, 1023 - j, H + j)
    Wzp = np.zeros((M_P1, DEG), dtype=np.float64)
    np.add.at(Wzp, (idx, col), 1.0 - t)
    np.add.at(Wzp, (idx + 1, col), t)

    # row 2048 feeds the leading z' columns (nodes sharing the last grid
    # interval); handled as a rank-1 DVE fix, so it must be a prefix run.
    nz2048 = np.nonzero(Wzp[2048])[0]
    fixn = int(nz2048.max()) + 1 if len(nz2048) else 1
    assert fixn < H and nz2048.tolist() == list(range(len(nz2048))), nz2048
    fixn += fixn % 2  # even width for 8B-aligned PSUM reads
    w2048 = np.ascontiguousarray(
        np.broadcast_to(Wzp[2048, :fixn].astype(np.float32), (128, fixn))
    ).copy()

    # bands per 128-row grid tile: contiguous col range within one PSUM bank
    bands = []
    packs = []
    off = 0
    for kt in range(NKT):
        blk = Wzp[kt * 128 : (kt + 1) * 128]
        cols = np.nonzero(np.abs(blk).max(axis=0) > 0)[0]
        lo, hi = int(cols.min()), int(cols.max())
        assert len(cols) == hi - lo + 1, f"band kt={kt} not contiguous"
        bank = lo // H
        assert hi // H == bank, f"band kt={kt} straddles banks"
        # even-align lo and width (PSUM 8B cachelines), stay inside the bank
        lo_e = lo - (lo % 2)
        n_e = hi - lo_e + 1
        n_e += n_e % 2
        if lo_e + n_e > (bank + 1) * H:
            lo_e -= 2
            assert lo_e >= bank * H
        bands.append((lo_e, n_e, off))
        packs.append(blk[:, lo_e : lo_e + n_e])
        off += n_e
    cw = off
    wz_packed = np.ascontiguousarray(
        np.concatenate(packs, axis=1).astype(BF16)
    )  # [128, cw]

    norm = ((2.0 - (np.arange(DEG) == 0)) / DEG).astype(np.float64)
    c = np.arange(H, dtype=np.float64)
    phi = np.pi * (c[:, None] + 0.5) / DEG  # [c, 1]
    e = np.arange(H, dtype=np.float64)[None, :]
    Be = norm[::2][None, :] * np.cos(2.0 * e * phi)        # [c, e] even coeffs
    Bo = norm[1::2][None, :] * np.cos((2.0 * e + 1.0) * phi)  # [c, o] odd

    def dev_layout(B):  # [512, 512] -> [128, 4, 512] (partition-major tiles)
        return np.ascontiguousarray(
            B.reshape(4, 128, H).transpose(1, 0, 2).astype(BF16)
        )

    return {
        "bands": tuple(bands),
        "cw": cw,
        "fixn": fixn,
        "w2048": w2048,
        "wz": wz_packed,
        "be": dev_layout(Be),
        "bo": dev_layout(Bo),
    }


def build_cheb_kernel(
    tc, y_ap, wz_ap, be_ap, bo_ap, id_ap, w48_ap, o_ap, rows, bands, fixn
):
    import concourse.mybir as mybir

    nc = tc.nc
    f32 = mybir.dt.float32
    bf16 = mybir.dt.bfloat16
    nblocks = rows // RB
    cw = sum(n for _, n, _ in bands)

    bank_of = [lo // H for lo, _, _ in bands]
    firsts = {bk: min(kt for kt in range(NKT) if bank_of[kt] == bk) for bk in (0, 1)}
    lasts = {bk: max(kt for kt in range(NKT) if bank_of[kt] == bk) for bk in (0, 1)}

    with (
        tc.tile_pool(name="consts", bufs=1) as consts,
        tc.tile_pool(name="ypool", bufs=3) as ypool,
        tc.tile_pool(name="ytpool", bufs=2) as ytpool,
        tc.tile_pool(name="uvpool", bufs=2) as uvpool,
        tc.tile_pool(name="utpool", bufs=2) as utpool,
        tc.tile_pool(name="opool", bufs=3) as opool,
        tc.tile_pool(name="fpool", bufs=6) as fpool,
        tc.tile_pool(name="zbpool", bufs=2) as zbpool,
        tc.tile_pool(name="zp", bufs=2, space="PSUM") as zpool,
        tc.tile_pool(name="pst", bufs=2, space="PSUM") as pstp,
        tc.tile_pool(name="ps2", bufs=1, space="PSUM") as ps2p,
    ):
        ident = consts.tile([128, 128], bf16)
        nc.sync.dma_start(out=ident, in_=id_ap)
        w48_sb = consts.tile([128, fixn], f32)
        nc.scalar.dma_start(out=w48_sb, in_=w48_ap)
        wz_sb = consts.tile([128, cw], bf16)
        nc.scalar.dma_start(out=wz_sb, in_=wz_ap)
        be_sb = consts.tile([128, 4, H], bf16)
        bo_sb = consts.tile([128, 4, H], bf16)
        for jj in range(4):
            eng = nc.sync if jj % 2 == 0 else nc.scalar
            eng.dma_start(out=be_sb[:, jj, :], in_=be_ap[:, jj, :])
            eng.dma_start(out=bo_sb[:, jj, :], in_=bo_ap[:, jj, :])

        ybs, ytbs, zps, us, vs, uts = {}, {}, {}, {}, {}, {}

        def load_y(b, split=False):
            yb = ypool.tile([128, M_P1], bf16, name="yb", tag="yb")
            rows_ap = y_ap[b * RB : (b + 1) * RB, :]
            if split:
                nc.sync.dma_start(out=yb[:, 0:1024], in_=rows_ap[:, 0:1024])
                nc.sync.dma_start(out=yb[:, 1024:M_P1], in_=rows_ap[:, 1024:M_P1])
            else:
                nc.sync.dma_start(out=yb, in_=rows_ap)
            ybs[b] = yb

        def emit_T(b):
            ytb = ytpool.tile([128, NKT, 128], bf16, name="ytb", tag="ytb")
            ytbs[b] = ytb
            for g in range(4):
                pst = pstp.tile([128, 4, 128], bf16, name="pst", tag="pst")
                for jj in range(4):
                    kk = g * 4 + jj
                    nc.tensor.transpose(
                        pst[:, jj, :], ybs[b][:, kk * 128 : (kk + 1) * 128], ident
                    )
                dst = ytb[:, g * 4 : (g + 1) * 4, :]
                if g % 2 == 0:
                    nc.vector.tensor_copy(dst, pst)
                else:
                    nc.scalar.copy(dst, pst)

        def emit_S1(b):
            zp = zpool.tile([128, DEG], f32, name="zp", tag="zp")
            zps[b] = zp
            for kt in range(NKT):
                lo, n, off = bands[kt]
                bk = bank_of[kt]
                nc.tensor.matmul(
                    zp[:, lo : lo + n],
                    ytbs[b][:, kt, :],
                    wz_sb[:, off : off + n],
                    start=(kt == firsts[bk]),
                    stop=(kt == lasts[bk]),
                    skip_group_check=True,
                )

        def emit_fold(b):
            zp = zps[b]
            # DVE reads at most one PSUM operand: stage upper z' half in SBUF
            zb = zbpool.tile([128, H], f32, name="zb", tag="zb")
            nc.scalar.copy(zb, zp[:, H:DEG])
            # rank-1 fix for grid row 2048 onto the leading z' columns
            ycol = fpool.tile([128, 1], f32, name="ycol", tag="ycol")
            nc.vector.tensor_copy(ycol, ybs[b][:, 2048:2049])
            tmpf = fpool.tile([128, fixn], f32, name="tmpf", tag="tmpf")
            nc.vector.tensor_scalar_mul(tmpf, w48_sb, ycol)
            za0 = fpool.tile([128, fixn], f32, name="za0", tag="za0")
            nc.vector.tensor_add(za0, zp[:, 0:fixn], tmpf)
            u = uvpool.tile([128, H], bf16, name="u", tag="u")
            v = uvpool.tile([128, H], bf16, name="v", tag="v")
            nc.vector.tensor_add(u, zp[:, 0:H], zb)
            nc.vector.tensor_sub(v, zp[:, 0:H], zb)
            nc.vector.tensor_add(u[:, 0:fixn], za0, zb[:, 0:fixn])
            nc.vector.tensor_sub(v[:, 0:fixn], za0, zb[:, 0:fixn])
            us[b], vs[b] = u, v
            del zps[b], ybs[b]

        def emit_T2(b):
            ut = utpool.tile([128, 8, 128], bf16, name="ut", tag="ut")
            uts[b] = ut
            for g, src in enumerate((us[b], vs[b])):
                pst = pstp.tile([128, 4, 128], bf16, name="pst2", tag="pst")
                for jj in range(4):
                    nc.tensor.transpose(
                        pst[:, jj, :], src[:, jj * 128 : (jj + 1) * 128], ident
                    )
                dst = ut[:, g * 4 : (g + 1) * 4, :]
                if g == 0:
                    nc.vector.tensor_copy(dst, pst)
                else:
                    nc.scalar.copy(dst, pst)
            del us[b], vs[b]

        def emit_S2(b):
            ut = uts[b]
            pse = ps2p.tile([128, H], f32, name="pse", tag="pse")
            pso = ps2p.tile([128, H], f32, name="pso", tag="pso")
            for jj in range(4):
                nc.tensor.matmul(
                    pse, ut[:, jj, :], be_sb[:, jj, :],
                    start=(jj == 0), stop=(jj == 3),
                )
            for jj in range(4):
                nc.tensor.matmul(
                    pso, ut[:, 4 + jj, :], bo_sb[:, jj, :],
                    start=(jj == 0), stop=(jj == 3),
                )
            osb = opool.tile([128, DEG], f32, name="osb", tag="osb")
            nc.scalar.copy(osb[:, 0::2], pse)
            nc.scalar.copy(osb[:, 1::2], pso)
            nc.sync.dma_start(out=o_ap[b * RB : (b + 1) * RB, :], in_=osb)
            del uts[b]

        load_y(0, split=True)
        for b in range(nblocks):
            if b + 1 < nblocks:
                load_y(b + 1)
            emit_T(b)
            emit_S1(b)
            emit_fold(b)
            if b >= 1:
                emit_T2(b - 1)
                emit_S2(b - 1)
        emit_T2(nblocks - 1)
        emit_S2(nblocks - 1)


def _build_nc(rows, bands, fixn, cw):
    import concourse.mybir as mybir
    import concourse.tile as tile
    from concourse import bacc

    f32 = mybir.dt.float32
    bf16 = mybir.dt.bfloat16
    nc = bacc.Bacc(
        "TRN2",
        target_bir_lowering=False,
        debug=False,
        enable_asserts=False,
        num_devices=N_CORES,
    )
    y_ap = nc.dram_tensor("y", [rows, M_P1], bf16, kind="ExternalInput").ap()
    wz_ap = nc.dram_tensor("wz", [128, cw], bf16, kind="ExternalInput").ap()
    be_ap = nc.dram_tensor("be", [128, 4, H], bf16, kind="ExternalInput").ap()
    bo_ap = nc.dram_tensor("bo", [128, 4, H], bf16, kind="ExternalInput").ap()
    id_ap = nc.dram_tensor("ident", [128, 128], bf16, kind="ExternalInput").ap()
    w48_ap = nc.dram_tensor("w48", [128, fixn], f32, kind="ExternalInput").ap()
    o_ap = nc.dram_tensor("o", [rows, DEG], f32, kind="ExternalOutput").ap()
    with tile.TileContext(nc) as tc:
        build_cheb_kernel(
            tc, y_ap, wz_ap, be_ap, bo_ap, id_ap, w48_ap, o_ap, rows, bands, fixn
        )
    nc.compile()
    return nc


def kernel(x: np.ndarray, y: np.ndarray) -> np.ndarray:
    global LAST_RESULTS, _TABLES
    from concourse import bass_utils

    x = np.asarray(x, dtype=np.float32)
    y = np.asarray(y, dtype=np.float32)
    assert y.shape == (N_OBS, M_P1), y.shape

    if _TABLES is None or not np.array_equal(_TABLES.get("x"), x):
        _TABLES = _build_tables(x)
        _TABLES["x"] = x.copy()
    T = _TABLES

    key = (ROWS_PER_CORE, T["bands"], T["fixn"])
    if key not in _COMPILED:
        _COMPILED[key] = _build_nc(ROWS_PER_CORE, T["bands"], T["fixn"], T["cw"])
    nc = _COMPILED[key]

    y_bf = np.ascontiguousarray(y.astype(BF16))
    ident = np.ascontiguousarray(np.eye(128, dtype=np.float32).astype(BF16))
    in_maps = [
        {
            "y": y_bf[i * ROWS_PER_CORE : (i + 1) * ROWS_PER_CORE],
            "wz": T["wz"],
            "be": T["be"],
            "bo": T["bo"],
            "ident": ident,
            "w48": T["w2048"],
        }
        for i in range(N_CORES)
    ]
    trace = bool(int(os.environ.get("CHEB_TRACE", "0")))
    res = bass_utils.run_bass_kernel_spmd(
        nc, in_maps, core_ids=list(range(N_CORES)), trace=trace
    )
    LAST_RESULTS = res
    out = np.concatenate([res.results[i]["o"] for i in range(N_CORES)], axis=0)
    return out.reshape(-1)
